# revision 17
# baseline (speedup 1.0000x reference)
"""Trainium2 Bass kernel for CausalSelfAttention with kron-structured bias and
column-masked causal attention.

Shapes (hardcoded): x (4,1024,512), H=8 heads, HD=64, attn_bias (8,64,64)
expanded by kron(ones(8,8)) onto the top-left 512x512 of the (1024,1024)
score matrix. Causal tril mask with every 16th column (j%16==15) zeroed.

Sharding: 8 cores = 4 batches x 2 head-groups (4 heads each), SPMD.

Per-core structure (all matmul data bf16; PSUM accumulation fp32):
  - K^T/Q^T projections (feature-major) into per-head "stack" tiles [128,T]:
    even heads hold K/Q rows 0:64 and E/BMQ rows 64:128, odd heads swapped,
    so every PSUM->SBUF move is partition-aligned. bk is dropped entirely
    (Q.bk is constant per query => softmax-invariant); bq is added during the
    Q move (DVE tensor_scalar_add), K moves on the ACT engine (scalar.copy).
  - The kron bias is folded into the S matmul as a rank-64 extension of the
    contraction: S^T = [K_h; E]^T.T @ [Q_h; BMQ_h] in ONE 128-contract matmul
    for blk0; blk1 uses the 64-contract K/Q rows only.
  - Key tiles are PAIRED into [128,1024] two-bank PSUM tiles (contiguous
    column packing) so ONE ACT exp instruction covers two tiles; per-partition
    -1e30 bias kills masked keys. Diagonal 128-col blocks masked by a bf16
    TRI multiply on GpSimd. Emission is software-pipelined: the next pair's
    S matmuls run on the PE while the previous pair's exp is on ACT.
  - V_ext (token-major, 65 cols/head: 64 feats + ones col for the softmax
    denominator) built with bias added via a partition-broadcast tile.
    O^T accumulates over key tiles; row 64 = denominator.
  - Normalization: DVE reciprocal_approx_fast on the den row -> GpSimd
    partition_broadcast -> DVE multiply into y^T (bf16).
  - Z = y^T.T @ Wp^T partial projection, bf16 out; host sums the two
    per-batch partials and adds bp.

DMA: tensors host-packed into fat-row tiles; dma_start issue cost (~0.6us
each on a sequencer) is split across the SP and ACT HWDGE issuers plus the
GpSimd SWDGE path, ordered by first-use time.
"""

import sys
import types

import numpy as np
import ml_dtypes

import concourse.bass as bass
import concourse.bacc as bacc
import concourse.tile as tile
from concourse import mybir
from concourse.bass_utils import run_bass_kernel_spmd


def _ensure_axon_hooks():
    """bass_utils' trace path imports antenv.axon_hooks unconditionally; some
    images lack that module. Provide it (and register the real NTFF hook when
    the axon boot shim is available) so tracing degrades gracefully."""
    try:
        import antenv.axon_hooks  # noqa: F401
        return
    except ImportError:
        pass
    m = types.ModuleType("antenv.axon_hooks")
    m._hook = None
    m.set_axon_ntff_profile_hook = lambda h: setattr(m, "_hook", h)
    m.get_axon_ntff_profile_hook = lambda: m._hook
    sys.modules["antenv.axon_hooks"] = m
    try:
        import antenv
        antenv.axon_hooks = m
    except ImportError:
        pass
    try:
        from trn_agent_boot.trn_boot import _ntff_profile_via_ctypes
        m.set_axon_ntff_profile_hook(
            _ntff_profile_via_ctypes("/opt/axon/libaxon_pjrt.so")
        )
    except Exception:
        pass


_ensure_axon_hooks()

F32 = mybir.dt.float32
BF16 = mybir.dt.bfloat16
AFT = mybir.ActivationFunctionType
BF = ml_dtypes.bfloat16

B, T, C, H = 4, 1024, 512, 8
HD = 64
SCALE = 1.0 / 8.0
GH = 4          # heads per core
N_CORES = 8

_CACHE = {}
LAST_RESULTS = None

# key-tile pairs per (blk): (jt, pair_base, width); c0 = 512 - width
_PAIRS0 = [[(0, 0, 512), (1, 512, 384)], [(2, 0, 256), (3, 256, 128)]]
_PAIRS1 = [[(0, 0, 512), (1, 512, 512)], [(2, 0, 512), (3, 512, 512)],
           [(4, 0, 512), (5, 512, 384)], [(6, 0, 256), (7, 256, 128)]]


def _kernel_body(tc, io):
    nc = tc.nc
    XP, WQP, WKP, WVP, WPP, EP, BMQP, TRIP, AUX, ONE1, WBX, Z = (
        io["XP"], io["WQP"], io["WKP"], io["WVP"], io["WPP"],
        io["EP"], io["BMQP"], io["TRIP"], io["AUX"], io["ONE1"], io["WBX"],
        io["Z"],
    )

    from contextlib import ExitStack
    with ExitStack() as ctx:
        const = ctx.enter_context(tc.tile_pool(name="const", bufs=1))
        pmm = ctx.enter_context(tc.tile_pool(name="pmm", bufs=2, space="PSUM"))
        ps = ctx.enter_context(tc.tile_pool(name="ps", bufs=2, space="PSUM"))
        pot = ctx.enter_context(tc.tile_pool(name="pot", bufs=2, space="PSUM"))
        spt = ctx.enter_context(tc.tile_pool(name="spt", bufs=4))
        sden = ctx.enter_context(tc.tile_pool(name="sden", bufs=2))
        sbc = ctx.enter_context(tc.tile_pool(name="sbc", bufs=2))
        szout = ctx.enter_context(tc.tile_pool(name="szout", bufs=3))

        def ctile(shape, tag, dt=BF16):
            return const.tile(shape, dt, tag=tag, name=tag)

        xt = ctile([128, 4096], "xt")
        wq = ctile([128, 1024], "wq")
        wk = ctile([128, 1024], "wk")
        wv = ctile([128, 1040], "wv")
        wp = ctile([128, 1024], "wp")
        kst = [ctile([128, T], f"kst{i}") for i in range(GH)]
        qst = [ctile([128, T], f"qst{i}") for i in range(GH)]
        ve = [ctile([128, 260], f"ve{i}") for i in range(8)]
        yt = [ctile([128, T], f"yt{i}") for i in range(2)]
        tri = ctile([128, 128], "tri")
        aux = ctile([128, 3], "aux", F32)
        one1 = ctile([1, 128], "one1")
        wbx = ctile([1, 260], "wbx")

        # ---- loads, split across issuers and ordered by first-use time.
        def chunk_p(eng, dst, src, n):
            p = dst.shape[0] // n
            for cp in range(n):
                eng.dma_start(out=dst[cp * p:(cp + 1) * p, :],
                              in_=src[cp * p:(cp + 1) * p, :])

        # SP: wk first (ACT's stream is delayed by its lazy act-table load),
        # interleaved with x ib0 ct-major; ACT gets wq + x ib1.
        for cf in range(2):
            for cp in range(2):
                nc.sync.dma_start(
                    out=wk[cp * 64:(cp + 1) * 64, cf * 512:(cf + 1) * 512],
                    in_=WKP[cp * 64:(cp + 1) * 64, cf * 512:(cf + 1) * 512])
        for cf in range(2):
            for cp in range(2):
                nc.scalar.dma_start(
                    out=wq[cp * 64:(cp + 1) * 64, cf * 512:(cf + 1) * 512],
                    in_=WQP[cp * 64:(cp + 1) * 64, cf * 512:(cf + 1) * 512])
        for ct in range(4):
            lo = ct * 1024
            for cp in range(2):
                nc.sync.dma_start(
                    out=xt[cp * 64:(cp + 1) * 64, lo:lo + 512],
                    in_=XP[cp * 64:(cp + 1) * 64, lo:lo + 512])
        for ct in range(4):
            lo = ct * 1024 + 512
            nc.scalar.dma_start(out=xt[:, lo:lo + 512], in_=XP[:, lo:lo + 512])
        for hp in range(GH):
            er = slice(64, 128) if hp % 2 == 0 else slice(0, 64)
            nc.sync.dma_start(out=kst[hp][er, 0:512], in_=EP[:, :])
            nc.sync.dma_start(out=qst[hp][er, 0:512],
                              in_=BMQP[:, 512 * hp:512 * (hp + 1)])
        nc.sync.dma_start(out=one1, in_=ONE1[:, :])
        nc.sync.dma_start(out=wbx, in_=WBX[:, :])
        chunk_p(nc.sync, wv, WVP, 4)
        chunk_p(nc.sync, tri, TRIP, 2)
        chunk_p(nc.sync, wp, WPP, 2)
        # GpSimd (SWDGE): aux
        chunk_p(nc.gpsimd, aux, AUX, 4)

        # ---- K^T / Q^T projections into per-head stacks
        def emit_proj(dt, ib):
            mmk = pmm.tile([128, 512], F32, tag="mm", name="mmk")
            for ct in range(4):
                nc.tensor.matmul(
                    mmk,
                    wk[:, ct * 256 + dt * 128: ct * 256 + (dt + 1) * 128],
                    xt[:, ct * 1024 + ib * 512: ct * 1024 + (ib + 1) * 512],
                    start=(ct == 0), stop=(ct == 3),
                )
            for half in range(2):
                hp = 2 * dt + half
                rows = slice(0, 64) if half == 0 else slice(64, 128)
                nc.scalar.copy(
                    kst[hp][rows, ib * 512:(ib + 1) * 512], mmk[rows, :]
                )
            mmq = pmm.tile([128, 512], F32, tag="mm", name="mmq")
            for ct in range(4):
                nc.tensor.matmul(
                    mmq,
                    wq[:, ct * 256 + dt * 128: ct * 256 + (dt + 1) * 128],
                    xt[:, ct * 1024 + ib * 512: ct * 1024 + (ib + 1) * 512],
                    start=(ct == 0), stop=(ct == 3),
                )
            for half in range(2):
                hp = 2 * dt + half
                rows = slice(0, 64) if half == 0 else slice(64, 128)
                nc.vector.tensor_scalar_add(
                    qst[hp][rows, ib * 512:(ib + 1) * 512],
                    mmq[rows, :],
                    aux[rows, 1 + dt:2 + dt],
                )

        # ---- V_ext per 128-token tile (bias via broadcast add)
        def emit_v(jt):
            vp = pmm.tile([128, 512], F32, tag="mm", name="vp")
            for ct in range(4):
                nc.tensor.matmul(
                    vp[:, 0:260],
                    xt[:, ct * 1024 + jt * 128: ct * 1024 + (jt + 1) * 128],
                    wv[:, ct * 260:(ct + 1) * 260],
                    start=(ct == 0), stop=False,
                )
            nc.tensor.matmul(vp[:, 0:260], one1, wbx, start=False, stop=True)
            nc.scalar.copy(ve[jt], vp[:, 0:260])

        # ---- attention for one (blk, head): software-pipelined pairs.
        # The normalization chain of the PREVIOUS head (pending) is emitted
        # after this head's first pair so the Pool/DVE queue positions never
        # head-of-line-block the next head's tri muls / O matmuls.
        def emit_attn(blk, hp, pending=None, finale=False):
            q0 = blk * 512
            dt, off = hp // 2, (hp % 2) * 64
            krows = slice(0, 128) if blk == 0 else (
                slice(0, 64) if hp % 2 == 0 else slice(64, 128))
            pairs = _PAIRS0 if blk == 0 else _PAIRS1
            otp = pot.tile([65, 512], F32, tag="ot", name="otp")
            njt = 4 * (blk + 1)

            def emit_s_pair(pair):
                sp = ps.tile([128, 1024], F32, tag="s", name="sp")
                pt = spt.tile([128, 1024], BF16, tag="pt", name="pt")
                lo = pair[0][1]
                hi = pair[-1][1] + pair[-1][2]
                for jt, base, width in pair:
                    c0 = 512 - width
                    nc.tensor.matmul(
                        sp[:, base:base + width],
                        kst[hp][krows, jt * 128:(jt + 1) * 128],
                        qst[hp][krows, q0 + c0:q0 + 512],
                        start=True, stop=True,
                    )
                nc.scalar.activation(
                    pt[:, lo:hi], sp[:, lo:hi], AFT.Exp, bias=aux[:, 0:1]
                )
                for jt, base, width in pair:
                    if jt - 4 * blk >= 0:
                        nc.vector.tensor_mul(
                            pt[:, base:base + 128], pt[:, base:base + 128], tri
                        )
                return pt

            def emit_o_pair(pair, pt):
                for jt, base, width in pair:
                    c0 = 512 - width
                    nc.tensor.matmul(
                        otp[:, c0:],
                        ve[jt][:, 65 * hp:65 * hp + 65],
                        pt[:, base:base + width],
                        start=(jt == 0), stop=(jt == njt - 1),
                    )

            pts = [emit_s_pair(pairs[0])]
            if pending is not None:
                pending[0]()
            for i in range(1, len(pairs)):
                pts.append(emit_s_pair(pairs[i]))
                emit_o_pair(pairs[i - 1], pts[i - 1])
            if pending is not None:
                pending[1]()
            emit_o_pair(pairs[-1], pts[-1])

            bc = sbc.tile([64, 512], F32, tag="bc", name="bc")

            def den_a():
                draw = sden.tile([1, 512], F32, tag="draw", name="draw")
                nc.vector.tensor_copy(draw, otp[64:65, :])
                den = sden.tile([1, 512], F32, tag="den", name="den")
                nc.vector.reciprocal_approx_fast(den, draw)
                nc.gpsimd.partition_broadcast(bc, den[0:1, :])

            def den_b():
                nc.vector.tensor_mul(
                    yt[dt][off:off + 64, q0:q0 + 512], otp[0:64, :], bc
                )

            def den_finale():
                # halved chain: each 256-col half releases two Z tiles
                draw = sden.tile([1, 512], F32, tag="draw", name="draw")
                nc.vector.tensor_copy(draw, otp[64:65, :])
                den = sden.tile([1, 512], F32, tag="den", name="den")
                nc.vector.reciprocal_approx_fast(den, draw)
                nc.gpsimd.partition_broadcast(bc, den[0:1, :])
                for q in range(2):
                    cs = slice(q * 256, (q + 1) * 256)
                    nc.vector.tensor_mul(
                        yt[dt][off:off + 64, q0 + q * 256:q0 + (q + 1) * 256],
                        otp[0:64, cs], bc[0:64, cs],
                    )
                    emit_z(4 + 2 * q, chunks=4)
                    emit_z(5 + 2 * q, chunks=4)
            if finale:
                return den_finale
            return (den_a, den_b)

        # ---- partial projection Z tile
        def emit_z(it, chunks=2):
            zp = pmm.tile([128, 512], F32, tag="mm", name="zp")
            for ct in range(2):
                nc.tensor.matmul(
                    zp,
                    yt[ct][:, it * 128:(it + 1) * 128],
                    wp[:, ct * 512:(ct + 1) * 512],
                    start=(ct == 0), stop=(ct == 1),
                )
            zs = szout.tile([128, 512], BF16, tag="z", name="zs")
            nc.scalar.copy(zs, zp)
            p = 128 // chunks
            for cp in range(chunks):
                nc.sync.dma_start(
                    out=Z[it * 128 + cp * p: it * 128 + (cp + 1) * p, :],
                    in_=zs[cp * p:(cp + 1) * p, :],
                )

        # ---- schedule: V/Z filler between attention heads keeps the PE
        # dense while ACT chews on the exps; den chains ride one head behind.
        for dt in range(2):
            for ib in range(2):
                emit_proj(dt, ib)
        for jt in range(4):
            emit_v(jt)
        pend = emit_attn(0, 0)
        emit_v(4)
        emit_v(5)
        pend = emit_attn(0, 1, pend)
        emit_v(6)
        emit_v(7)
        pend = emit_attn(0, 2, pend)
        pend = emit_attn(0, 3, pend)
        pend = emit_attn(1, 0, pend)
        pend = emit_attn(1, 1, pend)
        emit_z(0)
        emit_z(1)
        pend = emit_attn(1, 2, pend)
        emit_z(2)
        emit_z(3)
        pend = emit_attn(1, 3, pend)
        pend[0]()
        pend[1]()
        for it in range(4, 8):
            emit_z(it)


def _build():
    nc = bacc.Bacc("TRN2", target_bir_lowering=False, debug=False,
                   num_devices=N_CORES)
    io = {}

    def din(name, shape, dt=BF16):
        io[name] = nc.dram_tensor(name, shape, dt, kind="ExternalInput").ap()

    din("XP", (128, 4096))
    din("WQP", (128, 1024))
    din("WKP", (128, 1024))
    din("WVP", (128, 1040))
    din("WPP", (128, 1024))
    din("ONE1", (1, 128))
    din("WBX", (1, 260))
    din("EP", (64, 512))
    din("BMQP", (64, 2048))
    din("TRIP", (128, 128))
    din("AUX", (128, 3), F32)
    io["Z"] = nc.dram_tensor("Z", (T, C), BF16, kind="ExternalOutput").ap()

    with tile.TileContext(nc) as tc:
        _kernel_body(tc, io)
    nc.compile()
    return nc


def _host_prep(x, attn_bias, Wq, bq, Wk, bk, Wv, bv, Wp, bp):
    """Build the 8 per-core input maps (packed, bf16)."""
    f = np.float32
    EPa = np.zeros((64, 512), f)
    for n in range(64):
        EPa[n, n * 8:(n + 1) * 8] = 1.0
    EPa = EPa.astype(BF)
    TRIP = (np.arange(128)[None, :] >= np.arange(128)[:, None]).astype(f).astype(BF)

    in_maps = []
    for core in range(N_CORES):
        b, g = core // 2, core % 2
        gs = slice(256 * g, 256 * (g + 1))
        XPa = (x[b].T.reshape(4, 128, T).transpose(1, 0, 2)
               .reshape(128, 4 * T).astype(BF))
        WQP = ((Wq[gs, :] * SCALE).T.reshape(4, 128, 256).transpose(1, 0, 2)
               .reshape(128, 1024).astype(BF))
        WKP = (Wk[gs, :].T.reshape(4, 128, 256).transpose(1, 0, 2)
               .reshape(128, 1024).astype(BF))
        WvE = np.zeros((C, 260), f)
        WBa = np.zeros((1, 260), f)
        for hp in range(GH):
            r = slice(256 * g + 64 * hp, 256 * g + 64 * hp + 64)
            WvE[:, 65 * hp:65 * hp + 64] = Wv[r, :].T
            WBa[0, 65 * hp:65 * hp + 64] = bv[r]
            WBa[0, 65 * hp + 64] = 1.0
        WVP = (WvE.reshape(4, 128, 260).transpose(1, 0, 2)
               .reshape(128, 1040).astype(BF))
        WPP = (Wp[:, gs].T.reshape(2, 128, C).transpose(1, 0, 2)
               .reshape(128, 1024).astype(BF))
        BMQP = np.zeros((64, 2048), f)
        for hp in range(GH):
            h = GH * g + hp
            BMQP[:, 512 * hp:512 * (hp + 1)] = np.repeat(attn_bias[h], 8, axis=0).T
        AUX = np.zeros((128, 3), f)
        AUX[15::16, 0] = -1e30
        AUX[:, 1] = bq[gs][:128] * SCALE
        AUX[:, 2] = bq[gs][128:] * SCALE
        in_maps.append({
            "XP": XPa, "WQP": WQP, "WKP": WKP, "WVP": WVP,
            "WPP": WPP, "EP": EPa, "ONE1": np.ones((1, 128), f).astype(BF),
            "WBX": WBa.astype(BF),
            "BMQP": BMQP.astype(BF), "TRIP": TRIP, "AUX": AUX,
        })
    return in_maps


def kernel(**inputs):
    global LAST_RESULTS
    if "nc" not in _CACHE:
        _CACHE["nc"] = _build()
    nc = _CACHE["nc"]

    in_maps = _host_prep(**{k: np.asarray(v) for k, v in inputs.items()})
    res = run_bass_kernel_spmd(nc, in_maps, core_ids=list(range(N_CORES)))
    LAST_RESULTS = res

    bp = np.asarray(inputs["bp"], np.float32)
    out = np.empty((B, T, C), np.float32)
    for b in range(B):
        out[b] = (np.asarray(res.results[2 * b]["Z"]).astype(np.float32)
                  + np.asarray(res.results[2 * b + 1]["Z"]).astype(np.float32)
                  + bp[None, :])
    return out


# revision 18
# speedup vs baseline: 1.0608x; 1.0608x over previous
"""Trainium2 Bass kernel for CausalSelfAttention with kron-structured bias and
column-masked causal attention.

Shapes (hardcoded): x (4,1024,512), H=8 heads, HD=64, attn_bias (8,64,64)
expanded by kron(ones(8,8)) onto the top-left 512x512 of the (1024,1024)
score matrix. Causal tril mask with every 16th column (j%16==15) zeroed.

Sharding: 8 cores = 4 batches x 2 head-groups (4 heads each), SPMD.

Per-core structure (all matmul data bf16; PSUM accumulation fp32):
  - K^T/Q^T projections (feature-major) into per-head "stack" tiles [128,T]:
    even heads hold K/Q rows 0:64 and E/BMQ rows 64:128, odd heads swapped,
    so every PSUM->SBUF move is partition-aligned. bk is dropped entirely
    (Q.bk is constant per query => softmax-invariant); bq is added during the
    Q move (DVE tensor_scalar_add), K moves on the ACT engine (scalar.copy).
  - The kron bias is folded into the S matmul as a rank-64 extension of the
    contraction: S^T = [K_h; E]^T.T @ [Q_h; BMQ_h] in ONE 128-contract matmul
    for blk0; blk1 uses the 64-contract K/Q rows only.
  - Key tiles are PAIRED into [128,1024] two-bank PSUM tiles (contiguous
    column packing) so ONE ACT exp instruction covers two tiles; per-partition
    -1e30 bias kills masked keys. Diagonal 128-col blocks masked by a bf16
    TRI multiply on GpSimd. Emission is software-pipelined: the next pair's
    S matmuls run on the PE while the previous pair's exp is on ACT.
  - V_ext (token-major, 65 cols/head: 64 feats + ones col for the softmax
    denominator) built with bias added via a partition-broadcast tile.
    O^T accumulates over key tiles; row 64 = denominator.
  - Normalization: DVE reciprocal_approx_fast on the den row -> GpSimd
    partition_broadcast -> DVE multiply into y^T (bf16).
  - Z = y^T.T @ Wp^T partial projection, bf16 out; host sums the two
    per-batch partials and adds bp.

DMA: tensors host-packed into fat-row tiles; dma_start issue cost (~0.6us
each on a sequencer) is split across the SP and ACT HWDGE issuers plus the
GpSimd SWDGE path, ordered by first-use time.
"""

import sys
import types

import numpy as np
import ml_dtypes

import concourse.bass as bass
import concourse.bacc as bacc
import concourse.tile as tile
from concourse import mybir
from concourse.bass_utils import run_bass_kernel_spmd


def _ensure_axon_hooks():
    """bass_utils' trace path imports antenv.axon_hooks unconditionally; some
    images lack that module. Provide it (and register the real NTFF hook when
    the axon boot shim is available) so tracing degrades gracefully."""
    try:
        import antenv.axon_hooks  # noqa: F401
        return
    except ImportError:
        pass
    m = types.ModuleType("antenv.axon_hooks")
    m._hook = None
    m.set_axon_ntff_profile_hook = lambda h: setattr(m, "_hook", h)
    m.get_axon_ntff_profile_hook = lambda: m._hook
    sys.modules["antenv.axon_hooks"] = m
    try:
        import antenv
        antenv.axon_hooks = m
    except ImportError:
        pass
    try:
        from trn_agent_boot.trn_boot import _ntff_profile_via_ctypes
        m.set_axon_ntff_profile_hook(
            _ntff_profile_via_ctypes("/opt/axon/libaxon_pjrt.so")
        )
    except Exception:
        pass


_ensure_axon_hooks()

F32 = mybir.dt.float32
BF16 = mybir.dt.bfloat16
AFT = mybir.ActivationFunctionType
BF = ml_dtypes.bfloat16

B, T, C, H = 4, 1024, 512, 8
HD = 64
SCALE = 1.0 / 8.0
GH = 4          # heads per core
N_CORES = 8

_CACHE = {}
LAST_RESULTS = None

# key-tile pairs per (blk): (jt, pair_base, width); c0 = 512 - width
_PAIRS0 = [[(0, 0, 512), (1, 512, 384)], [(2, 0, 256), (3, 256, 128)]]
_PAIRS1 = [[(0, 0, 512), (1, 512, 512)], [(2, 0, 512), (3, 512, 512)],
           [(4, 0, 512), (5, 512, 384)], [(6, 0, 256), (7, 256, 128)]]


def _kernel_body(tc, io):
    nc = tc.nc
    XP, WQP, WKP, WVP, WPP, EP, BMQP, TRIP, AUX, ONE1, WBX, Z = (
        io["XP"], io["WQP"], io["WKP"], io["WVP"], io["WPP"],
        io["EP"], io["BMQP"], io["TRIP"], io["AUX"], io["ONE1"], io["WBX"],
        io["Z"],
    )

    from contextlib import ExitStack
    with ExitStack() as ctx:
        const = ctx.enter_context(tc.tile_pool(name="const", bufs=1))
        pmm = ctx.enter_context(tc.tile_pool(name="pmm", bufs=2, space="PSUM"))
        ps = ctx.enter_context(tc.tile_pool(name="ps", bufs=2, space="PSUM"))
        pot = ctx.enter_context(tc.tile_pool(name="pot", bufs=2, space="PSUM"))
        spt = ctx.enter_context(tc.tile_pool(name="spt", bufs=4))
        sden = ctx.enter_context(tc.tile_pool(name="sden", bufs=2))
        sbc = ctx.enter_context(tc.tile_pool(name="sbc", bufs=2))
        szout = ctx.enter_context(tc.tile_pool(name="szout", bufs=3))

        def ctile(shape, tag, dt=BF16):
            return const.tile(shape, dt, tag=tag, name=tag)

        xt = ctile([128, 4096], "xt")
        wq = ctile([128, 1024], "wq")
        wk = ctile([128, 1024], "wk")
        wv = ctile([128, 1040], "wv")
        wp = ctile([128, 1024], "wp")
        kst = [ctile([128, T], f"kst{i}") for i in range(GH)]
        qst = [ctile([128, T], f"qst{i}") for i in range(GH)]
        ve = [ctile([128, 260], f"ve{i}") for i in range(8)]
        yt = [ctile([128, T], f"yt{i}") for i in range(2)]
        tri = ctile([128, 128], "tri")
        aux = ctile([128, 3], "aux", F32)
        one1 = ctile([1, 128], "one1")
        wbx = ctile([1, 260], "wbx")

        # ---- loads, split across issuers and ordered by first-use time.
        def chunk_p(eng, dst, src, n):
            p = dst.shape[0] // n
            for cp in range(n):
                eng.dma_start(out=dst[cp * p:(cp + 1) * p, :],
                              in_=src[cp * p:(cp + 1) * p, :])

        # ACT: wk quarters first (gates the very first matmul), then wq.
        for cf in range(2):
            for cp in range(2):
                nc.scalar.dma_start(
                    out=wk[cp * 64:(cp + 1) * 64, cf * 512:(cf + 1) * 512],
                    in_=WKP[cp * 64:(cp + 1) * 64, cf * 512:(cf + 1) * 512])
        for cf in range(2):
            for cp in range(2):
                nc.scalar.dma_start(
                    out=wq[cp * 64:(cp + 1) * 64, cf * 512:(cf + 1) * 512],
                    in_=WQP[cp * 64:(cp + 1) * 64, cf * 512:(cf + 1) * 512])
        # SP: x ib0 ct-major (first proj group), then ib1, E/BMQ, wv, rest.
        for ct in range(4):
            lo = ct * 1024
            for cp in range(2):
                nc.sync.dma_start(
                    out=xt[cp * 64:(cp + 1) * 64, lo:lo + 512],
                    in_=XP[cp * 64:(cp + 1) * 64, lo:lo + 512])
        for ct in range(4):
            lo = ct * 1024 + 512
            nc.sync.dma_start(out=xt[:, lo:lo + 512], in_=XP[:, lo:lo + 512])
        for hp in range(GH):
            er = slice(64, 128) if hp % 2 == 0 else slice(0, 64)
            nc.sync.dma_start(out=kst[hp][er, 0:512], in_=EP[:, :])
            nc.sync.dma_start(out=qst[hp][er, 0:512],
                              in_=BMQP[:, 512 * hp:512 * (hp + 1)])
        nc.sync.dma_start(out=one1, in_=ONE1[:, :])
        nc.sync.dma_start(out=wbx, in_=WBX[:, :])
        chunk_p(nc.sync, wv, WVP, 4)
        chunk_p(nc.sync, tri, TRIP, 2)
        chunk_p(nc.sync, wp, WPP, 2)
        # GpSimd (SWDGE): aux
        chunk_p(nc.gpsimd, aux, AUX, 4)

        # ---- K^T / Q^T projections into per-head stacks
        def emit_proj(dt, ib):
            mmk = pmm.tile([128, 512], F32, tag="mm", name="mmk")
            for ct in range(4):
                nc.tensor.matmul(
                    mmk,
                    wk[:, ct * 256 + dt * 128: ct * 256 + (dt + 1) * 128],
                    xt[:, ct * 1024 + ib * 512: ct * 1024 + (ib + 1) * 512],
                    start=(ct == 0), stop=(ct == 3),
                )
            for half in range(2):
                hp = 2 * dt + half
                rows = slice(0, 64) if half == 0 else slice(64, 128)
                nc.scalar.copy(
                    kst[hp][rows, ib * 512:(ib + 1) * 512], mmk[rows, :]
                )
            mmq = pmm.tile([128, 512], F32, tag="mm", name="mmq")
            for ct in range(4):
                nc.tensor.matmul(
                    mmq,
                    wq[:, ct * 256 + dt * 128: ct * 256 + (dt + 1) * 128],
                    xt[:, ct * 1024 + ib * 512: ct * 1024 + (ib + 1) * 512],
                    start=(ct == 0), stop=(ct == 3),
                )
            for half in range(2):
                hp = 2 * dt + half
                rows = slice(0, 64) if half == 0 else slice(64, 128)
                nc.vector.tensor_scalar_add(
                    qst[hp][rows, ib * 512:(ib + 1) * 512],
                    mmq[rows, :],
                    aux[rows, 1 + dt:2 + dt],
                )

        # ---- V_ext per 128-token tile (bias via broadcast add)
        def emit_v(jt):
            vp = pmm.tile([128, 512], F32, tag="mm", name="vp")
            for ct in range(4):
                nc.tensor.matmul(
                    vp[:, 0:260],
                    xt[:, ct * 1024 + jt * 128: ct * 1024 + (jt + 1) * 128],
                    wv[:, ct * 260:(ct + 1) * 260],
                    start=(ct == 0), stop=False,
                )
            nc.tensor.matmul(vp[:, 0:260], one1, wbx, start=False, stop=True)
            nc.scalar.copy(ve[jt], vp[:, 0:260])

        # ---- attention for one (blk, head): software-pipelined pairs.
        # The normalization chain of the PREVIOUS head (pending) is emitted
        # after this head's first pair so the Pool/DVE queue positions never
        # head-of-line-block the next head's tri muls / O matmuls.
        def emit_attn(blk, hp, pending=None, finale=False):
            q0 = blk * 512
            dt, off = hp // 2, (hp % 2) * 64
            krows = slice(0, 128) if blk == 0 else (
                slice(0, 64) if hp % 2 == 0 else slice(64, 128))
            pairs = _PAIRS0 if blk == 0 else _PAIRS1
            otp = pot.tile([65, 512], F32, tag="ot", name="otp")
            njt = 4 * (blk + 1)

            def emit_s_pair(pair):
                sp = ps.tile([128, 1024], F32, tag="s", name="sp")
                pt = spt.tile([128, 1024], BF16, tag="pt", name="pt")
                lo = pair[0][1]
                hi = pair[-1][1] + pair[-1][2]
                for jt, base, width in pair:
                    c0 = 512 - width
                    nc.tensor.matmul(
                        sp[:, base:base + width],
                        kst[hp][krows, jt * 128:(jt + 1) * 128],
                        qst[hp][krows, q0 + c0:q0 + 512],
                        start=True, stop=True,
                    )
                nc.scalar.activation(
                    pt[:, lo:hi], sp[:, lo:hi], AFT.Exp, bias=aux[:, 0:1]
                )
                for jt, base, width in pair:
                    if jt - 4 * blk >= 0:
                        nc.vector.tensor_mul(
                            pt[:, base:base + 128], pt[:, base:base + 128], tri
                        )
                return pt

            def emit_o_pair(pair, pt):
                for jt, base, width in pair:
                    c0 = 512 - width
                    nc.tensor.matmul(
                        otp[:, c0:],
                        ve[jt][:, 65 * hp:65 * hp + 65],
                        pt[:, base:base + width],
                        start=(jt == 0), stop=(jt == njt - 1),
                    )

            pts = [emit_s_pair(pairs[0])]
            if pending is not None:
                pending[0]()
            for i in range(1, len(pairs)):
                pts.append(emit_s_pair(pairs[i]))
                emit_o_pair(pairs[i - 1], pts[i - 1])
            if pending is not None:
                pending[1]()
            emit_o_pair(pairs[-1], pts[-1])

            bc = sbc.tile([64, 512], F32, tag="bc", name="bc")

            def den_a():
                draw = sden.tile([1, 512], F32, tag="draw", name="draw")
                nc.vector.tensor_copy(draw, otp[64:65, :])
                den = sden.tile([1, 512], F32, tag="den", name="den")
                nc.vector.reciprocal_approx_fast(den, draw)
                nc.gpsimd.partition_broadcast(bc, den[0:1, :])

            def den_b():
                nc.vector.tensor_mul(
                    yt[dt][off:off + 64, q0:q0 + 512], otp[0:64, :], bc
                )

            def den_finale():
                # halved chain: each 256-col half releases two Z tiles
                draw = sden.tile([1, 512], F32, tag="draw", name="draw")
                nc.vector.tensor_copy(draw, otp[64:65, :])
                den = sden.tile([1, 512], F32, tag="den", name="den")
                nc.vector.reciprocal_approx_fast(den, draw)
                nc.gpsimd.partition_broadcast(bc, den[0:1, :])
                for q in range(2):
                    cs = slice(q * 256, (q + 1) * 256)
                    nc.vector.tensor_mul(
                        yt[dt][off:off + 64, q0 + q * 256:q0 + (q + 1) * 256],
                        otp[0:64, cs], bc[0:64, cs],
                    )
                    emit_z(4 + 2 * q, chunks=4)
                    emit_z(5 + 2 * q, chunks=4)
            if finale:
                return den_finale
            return (den_a, den_b)

        # ---- partial projection Z tile
        def emit_z(it, chunks=2):
            zp = pmm.tile([128, 512], F32, tag="mm", name="zp")
            for ct in range(2):
                nc.tensor.matmul(
                    zp,
                    yt[ct][:, it * 128:(it + 1) * 128],
                    wp[:, ct * 512:(ct + 1) * 512],
                    start=(ct == 0), stop=(ct == 1),
                )
            zs = szout.tile([128, 512], BF16, tag="z", name="zs")
            nc.scalar.copy(zs, zp)
            p = 128 // chunks
            for cp in range(chunks):
                nc.sync.dma_start(
                    out=Z[it * 128 + cp * p: it * 128 + (cp + 1) * p, :],
                    in_=zs[cp * p:(cp + 1) * p, :],
                )

        # ---- schedule: V/Z filler between attention heads keeps the PE
        # dense while ACT chews on the exps; den chains ride one head behind.
        for dt in range(2):
            for ib in range(2):
                emit_proj(dt, ib)
        for jt in range(4):
            emit_v(jt)
        pend = emit_attn(0, 0)
        emit_v(4)
        emit_v(5)
        pend = emit_attn(0, 1, pend)
        emit_v(6)
        emit_v(7)
        pend = emit_attn(0, 2, pend)
        pend = emit_attn(0, 3, pend)
        pend = emit_attn(1, 0, pend)
        pend = emit_attn(1, 1, pend)
        emit_z(0)
        emit_z(1)
        pend = emit_attn(1, 2, pend)
        emit_z(2)
        emit_z(3)
        pend = emit_attn(1, 3, pend)
        pend[0]()
        pend[1]()
        for it in range(4, 8):
            emit_z(it)


def _build():
    nc = bacc.Bacc("TRN2", target_bir_lowering=False, debug=False,
                   num_devices=N_CORES)
    io = {}

    def din(name, shape, dt=BF16):
        io[name] = nc.dram_tensor(name, shape, dt, kind="ExternalInput").ap()

    din("XP", (128, 4096))
    din("WQP", (128, 1024))
    din("WKP", (128, 1024))
    din("WVP", (128, 1040))
    din("WPP", (128, 1024))
    din("ONE1", (1, 128))
    din("WBX", (1, 260))
    din("EP", (64, 512))
    din("BMQP", (64, 2048))
    din("TRIP", (128, 128))
    din("AUX", (128, 3), F32)
    io["Z"] = nc.dram_tensor("Z", (T, C), BF16, kind="ExternalOutput").ap()

    with tile.TileContext(nc) as tc:
        _kernel_body(tc, io)
    nc.compile()
    return nc


def _host_prep(x, attn_bias, Wq, bq, Wk, bk, Wv, bv, Wp, bp):
    """Build the 8 per-core input maps (packed, bf16)."""
    f = np.float32
    EPa = np.zeros((64, 512), f)
    for n in range(64):
        EPa[n, n * 8:(n + 1) * 8] = 1.0
    EPa = EPa.astype(BF)
    TRIP = (np.arange(128)[None, :] >= np.arange(128)[:, None]).astype(f).astype(BF)

    in_maps = []
    for core in range(N_CORES):
        b, g = core // 2, core % 2
        gs = slice(256 * g, 256 * (g + 1))
        XPa = (x[b].T.reshape(4, 128, T).transpose(1, 0, 2)
               .reshape(128, 4 * T).astype(BF))
        WQP = ((Wq[gs, :] * SCALE).T.reshape(4, 128, 256).transpose(1, 0, 2)
               .reshape(128, 1024).astype(BF))
        WKP = (Wk[gs, :].T.reshape(4, 128, 256).transpose(1, 0, 2)
               .reshape(128, 1024).astype(BF))
        WvE = np.zeros((C, 260), f)
        WBa = np.zeros((1, 260), f)
        for hp in range(GH):
            r = slice(256 * g + 64 * hp, 256 * g + 64 * hp + 64)
            WvE[:, 65 * hp:65 * hp + 64] = Wv[r, :].T
            WBa[0, 65 * hp:65 * hp + 64] = bv[r]
            WBa[0, 65 * hp + 64] = 1.0
        WVP = (WvE.reshape(4, 128, 260).transpose(1, 0, 2)
               .reshape(128, 1040).astype(BF))
        WPP = (Wp[:, gs].T.reshape(2, 128, C).transpose(1, 0, 2)
               .reshape(128, 1024).astype(BF))
        BMQP = np.zeros((64, 2048), f)
        for hp in range(GH):
            h = GH * g + hp
            BMQP[:, 512 * hp:512 * (hp + 1)] = np.repeat(attn_bias[h], 8, axis=0).T
        AUX = np.zeros((128, 3), f)
        AUX[15::16, 0] = -1e30
        AUX[:, 1] = bq[gs][:128] * SCALE
        AUX[:, 2] = bq[gs][128:] * SCALE
        in_maps.append({
            "XP": XPa, "WQP": WQP, "WKP": WKP, "WVP": WVP,
            "WPP": WPP, "EP": EPa, "ONE1": np.ones((1, 128), f).astype(BF),
            "WBX": WBa.astype(BF),
            "BMQP": BMQP.astype(BF), "TRIP": TRIP, "AUX": AUX,
        })
    return in_maps


def kernel(**inputs):
    global LAST_RESULTS
    if "nc" not in _CACHE:
        _CACHE["nc"] = _build()
    nc = _CACHE["nc"]

    in_maps = _host_prep(**{k: np.asarray(v) for k, v in inputs.items()})
    res = run_bass_kernel_spmd(nc, in_maps, core_ids=list(range(N_CORES)))
    LAST_RESULTS = res

    bp = np.asarray(inputs["bp"], np.float32)
    out = np.empty((B, T, C), np.float32)
    for b in range(B):
        out[b] = (np.asarray(res.results[2 * b]["Z"]).astype(np.float32)
                  + np.asarray(res.results[2 * b + 1]["Z"]).astype(np.float32)
                  + bp[None, :])
    return out
